# revision 3
# baseline (speedup 1.0000x reference)
"""EventDenoisingMamba Trainium2 kernel (v2).

Data-parallel over batch: 8 batch elements -> 8 NeuronCores, one full
sequence (S=8192) per core. On chip: channels on partitions (2 blocks of
128 for d_inner=256, db-major along the free dim), time on the free dim.

v2 changes vs v1:
  - x activations live in DRAM ping-ping buffers (one per layer); each
    chunk loads a [128, L+3] input tile and stores a [128, L] output tile.
  - B/C broadcasts: xd rows bounce through DRAM once per chunk, then come
    back as FOUR grouped broadcast reads per side ([128, 4, L], 8KB per
    partition descriptors) instead of 32 per-row 2KB-descriptor DMAs.
  - Scan carries: the last h column of each (state, db) is copied by the
    Scalar engine into a small [128, 32] f32 carry tile (was: 512 strided
    DVE CASTs at 1.35us each); next chunk's scans use it as `initial`.
  - Elementwise mults/adds split DVE/GPSIMD by a tunable pattern.
  - ACT ops emitted silu-block-first then exp/ln-block to minimise
    activation-table switches.
"""

import numpy as np

import concourse.bass as bass
import concourse.tile as tile
from concourse import bacc, mybir

F32 = mybir.dt.float32
BF16 = mybir.dt.bfloat16
AF = mybir.ActivationFunctionType
OP = mybir.AluOpType

S = 8192
DM = 128      # d_model
DI = 256      # d_inner
NST = 16      # d_state
DC = 4        # d_conv
RK = 8        # dt_rank
NL = 4        # layers
NCORES = 8

# engine split tunables: which states' ops go to GPSIMD
DBU_GPS = {1, 3, 5, 7, 9, 11, 13, 15}
TM_GPS = {0, 2, 4, 6, 8, 10, 12, 14}
ADD_GPS = {2, 5, 6, 9, 10, 13, 14}


class Ctx:
    pass


def _bcast_view(t, r, L):
    """[128, 2, L] stride-0-partition-dup view of group tile row r."""
    return bass.AP(tensor=t.tensor, offset=t.offset + r * L,
                   ap=[list(t.ap[0]), [0, 2], [1, L]])


def _load_weights(c, nc, drams):
    wp = c.wp
    (kuc, wz, xpw, dtw, ow, emb, headw, dtb, cb, aneg, dpar, embb,
     headb) = drams
    c.w_kuc, c.w_wz, c.w_xpw, c.w_dtw, c.w_ow = [], [], [], [], []
    c.w_dtb, c.w_cb, c.w_a, c.w_d = [], [], [], []
    for l in range(NL):
        for lst, dram, shape, dt in [
            (c.w_kuc, kuc, [128, DC * DI], BF16),
            (c.w_wz, wz, [128, DI], BF16),
            (c.w_xpw, xpw, [128, 80], BF16),
            (c.w_dtw, dtw, [RK, DI], BF16),
            (c.w_ow, ow, [128, 256], BF16),
            (c.w_dtb, dtb, [128, 2], F32),
            (c.w_cb, cb, [128, 2], F32),
            (c.w_a, aneg, [128, 2 * NST], F32),
            (c.w_d, dpar, [128, 2], F32),
        ]:
            t = wp.tile(shape, dt, tag=f"w{len(lst)}_{id(lst) % 997}",
                        name=f"w{len(lst)}_{id(lst) % 997}")
            nc.sync.dma_start(t, dram[l])
            lst.append(t)
    c.w_emb = wp.tile([11, DM], BF16, tag="emb", name="emb")
    nc.sync.dma_start(c.w_emb, emb[:])
    c.w_headw = wp.tile([DM, 1], BF16, tag="headw", name="headw")
    nc.sync.dma_start(c.w_headw, headw[:])
    c.w_embb = wp.tile([128, 1], F32, tag="embb", name="embb")
    nc.sync.dma_start(c.w_embb, embb[:])
    c.w_headb = wp.tile([1, 1], F32, tag="headb", name="headb")
    nc.sync.dma_start(c.w_headb, headb[:])


def _embed(c, nc, featT, xbuf):
    L = c.L
    for blk in range(c.s // L):
        ps = c.pp.tile([128, L], F32, tag="mm", name="mm")
        for h2 in range(L // 512):
            col = blk * L + h2 * 512
            nc.tensor.matmul(
                ps[:, h2 * 512:(h2 + 1) * 512],
                lhsT=c.w_emb, rhs=c.w_feat[:, col:col + 512],
                start=True, stop=True)
        xo = c.xop.tile([128, L], BF16, tag="xo", name="xo")
        nc.scalar.activation(xo, ps, AF.Identity, bias=c.w_embb[:, 0:1])
        nc.sync.dma_start(xbuf[0][:, 3 + blk * L: 3 + (blk + 1) * L], xo)


def _chunk(c, nc, l, ci, xbuf, bcd):
    L = c.L
    t0 = ci * L

    xt = c.xtp.tile([128, L + 3], BF16, tag="xt", name="xt")
    nc.sync.dma_start(xt, xbuf[l][:, t0: t0 + L + 3])

    u = c.up.tile([128, 2 * L], BF16, tag="u", name="u")
    z = c.zp.tile([128, 2 * L], BF16, tag="z", name="z")
    de = c.dep.tile([128, 2 * L], BF16, tag="de", name="de")
    du = c.dup.tile([128, 2 * L], BF16, tag="du", name="du")

    # ---- u path (conv folded into 4 accumulated matmuls) + silu
    for db in range(2):
        ps = c.pp.tile([128, L], F32, tag="mm", name="mm")
        for h2 in range(L // 512):
            for k in range(DC):
                nc.tensor.matmul(
                    ps[:, h2 * 512:(h2 + 1) * 512],
                    lhsT=c.w_kuc[l][:, k * DI + db * 128:k * DI + db * 128 + 128],
                    rhs=xt[:, h2 * 512 + k: h2 * 512 + k + 512],
                    start=(k == 0), stop=(k == DC - 1))
        nc.scalar.activation(u[:, db * L:(db + 1) * L], ps, AF.Silu,
                             bias=c.w_cb[l][:, db:db + 1])
    # ---- z path + silu
    for db in range(2):
        ps = c.pp.tile([128, L], F32, tag="mm", name="mm")
        for h2 in range(L // 512):
            nc.tensor.matmul(
                ps[:, h2 * 512:(h2 + 1) * 512],
                lhsT=c.w_wz[l][:, db * 128:db * 128 + 128],
                rhs=xt[:, 3 + h2 * 512: 3 + h2 * 512 + 512],
                start=True, stop=True)
        nc.scalar.activation(z[:, db * L:(db + 1) * L], ps, AF.Silu)

    # ---- x_dbl = u @ xp_w -> [40, L]
    xd = c.xdp.tile([40, L], BF16, tag="xd", name="xd")
    ps = c.pp.tile([128, L], F32, tag="mm", name="mm")
    for h2 in range(L // 512):
        for ct in range(2):
            nc.tensor.matmul(
                ps[0:40, h2 * 512:(h2 + 1) * 512],
                lhsT=c.w_xpw[l][:, ct * 40:ct * 40 + 40],
                rhs=u[:, ct * L + h2 * 512: ct * L + h2 * 512 + 512],
                start=(ct == 0), stop=(ct == 1))
    nc.scalar.activation(xd, ps[0:40, :], AF.Copy)

    # ---- B/C rows to DRAM, back as 4-row grouped partition-broadcasts
    slot = (l * c.nch + ci) % 4
    nc.gpsimd.dma_start(bcd[slot], xd[RK:RK + 2 * NST, :])
    bgrp, cgrp = [], []
    for g in range(4):
        bb = c.bbp.tile([128, 4, L], BF16, tag="bb", name="bb")
        grp = bcd[slot][4 * g:4 * g + 4, :]
        src = bass.AP(tensor=grp.tensor, offset=grp.offset,
                      ap=[[0, 128]] + [list(x) for x in grp.ap])
        (nc.sync if g % 2 == 0 else nc.scalar).dma_start(bb, src)
        bgrp.append(bb)
    for g in range(4):
        cc = c.ccp.tile([128, 4, L], BF16, tag="cc", name="cc")
        grp = bcd[slot][NST + 4 * g:NST + 4 * g + 4, :]
        src = bass.AP(tensor=grp.tensor, offset=grp.offset,
                      ap=[[0, 128]] + [list(x) for x in grp.ap])
        (nc.scalar if g % 2 == 0 else nc.sync).dma_start(cc, src)
        cgrp.append(cc)

    # ---- delta = softplus(dt @ dtw + dtb), per db
    # softplus(x) = relu(x) + ln(1 + exp(-|x|))
    for db in range(2):
        ps = c.pp.tile([128, L], F32, tag="mm", name="mm")
        for h2 in range(L // 512):
            nc.tensor.matmul(
                ps[:, h2 * 512:(h2 + 1) * 512],
                lhsT=c.w_dtw[l][:, db * 128:db * 128 + 128],
                rhs=xd[0:RK, h2 * 512:h2 * 512 + 512],
                start=True, stop=True)
        ab = c.abp.tile([128, L], F32, tag="ab", name="ab")
        nc.scalar.activation(ab, ps, AF.Abs, bias=c.w_dtb[l][:, db:db + 1])
        nc.scalar.activation(de[:, db * L:(db + 1) * L], ps, AF.Relu,
                             bias=c.w_dtb[l][:, db:db + 1])
        ab2 = c.ab2p.tile([128, L], BF16, tag="ab2", name="ab2")
        nc.scalar.activation(ab2, ab, AF.Exp, scale=-1.0)
        nc.scalar.activation(ab2, ab2, AF.Ln, bias=1.0)
        nc.vector.tensor_tensor(de[:, db * L:(db + 1) * L],
                                de[:, db * L:(db + 1) * L], ab2, OP.add)

    nc.vector.tensor_tensor(du, de, u, OP.mult)

    # ---- selective scan over 16 states
    hc = c.hcp.tile([128, 2 * NST], F32, tag="hc", name="hc")
    acc = [None, None]
    de3 = de.rearrange("p (b t) -> p b t", b=2)
    du3 = du.rearrange("p (b t) -> p b t", b=2)
    for n in range(NST):
        g, r = n // 4, n % 4
        # da = exp(delta * A_n) per db (f32 for scan precision)
        das = []
        for db in range(2):
            da = c.dap.tile([128, L], F32, tag="da", name="da")
            nc.scalar.activation(
                da, de3[:, db, :], AF.Exp,
                scale=c.w_a[l][:, db * NST + n:db * NST + n + 1])
            das.append(da)
        # dbu = du * B_n (B broadcast across partitions, dup over db)
        dbu = c.dbup.tile([128, 2 * L], BF16, tag="dbu", name="dbu")
        deng = nc.gpsimd if n in DBU_GPS else nc.vector
        deng.tensor_tensor(dbu.rearrange("p (b t) -> p b t", b=2), du3,
                           _bcast_view(bgrp[g], r, L), OP.mult)
        # scan: h = da * h_prev + dbu, chained via carry tile
        h = c.hp.tile([128, 2 * L], BF16, tag="h", name="h")
        for db in range(2):
            init = 0.0 if ci == 0 else c.hc_prev[:, 2 * n + db:2 * n + db + 1]
            nc.vector.tensor_tensor_scan(
                h[:, db * L:(db + 1) * L], das[db],
                dbu[:, db * L:(db + 1) * L],
                initial=init, op0=OP.mult, op1=OP.add)
        # carry out last column of both db segments (Scalar engine copy)
        hsrc = bass.AP(tensor=h.tensor, offset=h.offset + L - 1,
                       ap=[list(h.ap[0]), [L, 2]])
        nc.scalar.activation(hc[:, 2 * n:2 * n + 2], hsrc, AF.Copy)
        # tm = h * C_n ; accumulate into acc[n % 2]
        teng = nc.gpsimd if n in TM_GPS else nc.vector
        if n < 2:
            a = c.accp.tile([128, 2 * L], BF16, tag=f"acc{n}", name="acc")
            teng.tensor_tensor(a.rearrange("p (b t) -> p b t", b=2),
                               h.rearrange("p (b t) -> p b t", b=2),
                               _bcast_view(cgrp[g], r, L), OP.mult)
            acc[n] = a
        else:
            tm = c.tmp.tile([128, 2 * L], BF16, tag="tm", name="tm")
            teng.tensor_tensor(tm.rearrange("p (b t) -> p b t", b=2),
                               h.rearrange("p (b t) -> p b t", b=2),
                               _bcast_view(cgrp[g], r, L), OP.mult)
            aeng = nc.gpsimd if n in ADD_GPS else nc.vector
            aeng.tensor_tensor(acc[n % 2], acc[n % 2], tm, OP.add)
    c.hc_prev = hc

    # ---- y = acc0 + acc1 + u*D, gated by silu(z)
    y = acc[0]
    nc.vector.tensor_tensor(y, y, acc[1], OP.add)
    for db in range(2):
        sl = slice(db * L, (db + 1) * L)
        nc.vector.scalar_tensor_tensor(
            y[:, sl], u[:, sl], c.w_d[l][:, db:db + 1], y[:, sl],
            OP.mult, OP.add)
    nc.vector.tensor_tensor(y, y, z, OP.mult)

    # ---- out_proj -> next layer's x buffer
    ps = c.pp.tile([128, L], F32, tag="mm", name="mm")
    for h2 in range(L // 512):
        for ct in range(2):
            nc.tensor.matmul(
                ps[:, h2 * 512:(h2 + 1) * 512],
                lhsT=c.w_ow[l][:, ct * 128:ct * 128 + 128],
                rhs=y[:, ct * L + h2 * 512: ct * L + h2 * 512 + 512],
                start=(ct == 0), stop=(ct == 1))
    xo = c.xop.tile([128, L], BF16, tag="xo", name="xo")
    nc.scalar.activation(xo, ps, AF.Copy)
    nc.scalar.dma_start(xbuf[l + 1][:, 3 + t0: 3 + t0 + L], xo)


def _head(c, nc, xbuf, out):
    L = c.L
    for blk in range(c.s // L):
        xt = c.xtp.tile([128, L + 3], BF16, tag="xt", name="xt")
        nc.sync.dma_start(xt, xbuf[NL][:, blk * L: blk * L + L + 3])
        ps = c.pp.tile([128, L], F32, tag="mm", name="mm")
        for h2 in range(L // 512):
            nc.tensor.matmul(
                ps[0:1, h2 * 512:(h2 + 1) * 512],
                lhsT=c.w_headw, rhs=xt[:, 3 + h2 * 512: 3 + h2 * 512 + 512],
                start=True, stop=True)
        ot = c.abp.tile([128, L], F32, tag="ab", name="ot")
        nc.scalar.activation(ot[0:1, :], ps[0:1, :], AF.Sigmoid,
                             bias=c.w_headb[0:1, 0:1])
        nc.sync.dma_start(out[0:1, blk * L:(blk + 1) * L], ot[0:1, :])


def build(s=S, tc_len=1024, nloops=1):
    nc = bacc.Bacc("TRN2", target_bir_lowering=False, debug=False,
                   num_devices=NCORES)
    drams = (
        nc.declare_dram_parameter("kuc", [NL, 128, DC * DI], BF16, False),
        nc.declare_dram_parameter("wz", [NL, 128, DI], BF16, False),
        nc.declare_dram_parameter("xpw", [NL, 128, 80], BF16, False),
        nc.declare_dram_parameter("dtw", [NL, RK, DI], BF16, False),
        nc.declare_dram_parameter("ow", [NL, 128, 256], BF16, False),
        nc.declare_dram_parameter("emb", [11, DM], BF16, False),
        nc.declare_dram_parameter("headw", [DM, 1], BF16, False),
        nc.declare_dram_parameter("dtb", [NL, 128, 2], F32, False),
        nc.declare_dram_parameter("cb", [NL, 128, 2], F32, False),
        nc.declare_dram_parameter("aneg", [NL, 128, 2 * NST], F32, False),
        nc.declare_dram_parameter("dpar", [NL, 128, 2], F32, False),
        nc.declare_dram_parameter("embb", [128, 1], F32, False),
        nc.declare_dram_parameter("headb", [1, 1], F32, False),
    )
    featT = nc.declare_dram_parameter("featT", [11, s], BF16, False)
    out = nc.declare_dram_parameter("out", [1, s], F32, True)
    bcd = nc.dram_tensor("bcd", [4, 2 * NST, tc_len], BF16)
    xbuf = [nc.dram_tensor(f"xb{i}", [128, 3 + s], BF16)
            for i in range(NL + 1)]

    c = Ctx()
    c.s = s
    c.L = tc_len
    c.nch = s // tc_len

    with tile.TileContext(nc) as tcx:
        with (
            tcx.tile_pool(name="w", bufs=1) as wp,
            tcx.tile_pool(name="psP", bufs=3, space="PSUM") as pp,
            tcx.tile_pool(name="xo", bufs=3) as xop,
            tcx.tile_pool(name="ab", bufs=2) as abp,
        ):
            c.wp, c.pp, c.xop, c.abp = wp, pp, xop, abp
            _load_weights(c, nc, drams)

            zt = wp.tile([128, 3], BF16, tag="zpad", name="zpad")
            nc.vector.memset(zt, 0.0)
            for i in range(NL + 1):
                nc.sync.dma_start(xbuf[i][:, 0:3], zt)

            with tcx.tile_pool(name="feat", bufs=1) as fp:
                c.w_feat = fp.tile([11, s], BF16, tag="featT", name="featT")
                nc.sync.dma_start(c.w_feat, featT[:])
                _embed(c, nc, featT, xbuf)

            from contextlib import ExitStack
            with ExitStack() as es:
                pools = {
                    "xtp": ("xt", 3), "up": ("u", 2), "zp": ("z", 2),
                    "dep": ("de", 2), "dup": ("du", 2), "xdp": ("xd", 2),
                    "bbp": ("bb", 2), "ccp": ("cc", 2), "dap": ("da", 4),
                    "dbup": ("dbu", 4), "hp": ("h", 6), "hcp": ("hcp", 2),
                    "tmp": ("tm", 4), "accp": ("accp", 2),
                    "ab2p": ("ab2", 2),
                }
                for attr, (nm, bufs) in pools.items():
                    setattr(c, attr,
                            es.enter_context(tcx.tile_pool(name=nm, bufs=bufs)))

                for rep in range(nloops):
                    for l in range(NL):
                        for ci in range(c.nch):
                            _chunk(c, nc, l, ci, xbuf, bcd)
                _head(c, nc, xbuf, out)

    nc.compile()
    return nc


_CACHE = {}


def _get_nc(s, tc_len, nloops=1):
    key = (s, tc_len, nloops)
    if key not in _CACHE:
        _CACHE[key] = build(s, tc_len, nloops)
    return _CACHE[key]


def prep_inputs(features, emb_w, emb_b, in_proj_w, conv_w, conv_b, x_proj_w,
                dt_w, dt_b, A_log, D, out_proj_w, head_w, head_b):
    """Host-side weight preprocessing shared by all cores."""
    import ml_dtypes
    f32 = np.float32
    bf16 = ml_dtypes.bfloat16

    nl = in_proj_w.shape[0]
    kuc = np.zeros((nl, 128, DC * DI), dtype=f32)
    for l in range(nl):
        wu = in_proj_w[l][:, :DI]                      # [128, 256]
        for k in range(DC):
            kuc[l][:, k * DI:(k + 1) * DI] = wu * conv_w[l][:, k][None, :]
    wz = in_proj_w[:, :, DI:]                          # [NL, 128, 256]
    xpw = np.zeros((nl, 128, 80), dtype=f32)
    ow = np.zeros((nl, 128, 256), dtype=f32)
    aneg = np.zeros((nl, 128, 2 * NST), dtype=f32)
    dtb2 = np.zeros((nl, 128, 2), dtype=f32)
    cb2 = np.zeros((nl, 128, 2), dtype=f32)
    dp2 = np.zeros((nl, 128, 2), dtype=f32)
    for l in range(nl):
        for ct in range(2):
            xpw[l][:, ct * 40:(ct + 1) * 40] = \
                x_proj_w[l][ct * 128:(ct + 1) * 128, :]
            ow[l][:, ct * 128:(ct + 1) * 128] = \
                out_proj_w[l][ct * 128:(ct + 1) * 128, :]
            aneg[l][:, ct * NST:(ct + 1) * NST] = \
                -np.exp(A_log[l][ct * 128:(ct + 1) * 128, :])
            dtb2[l][:, ct] = dt_b[l][ct * 128:(ct + 1) * 128]
            cb2[l][:, ct] = conv_b[l][ct * 128:(ct + 1) * 128]
            dp2[l][:, ct] = D[l][ct * 128:(ct + 1) * 128]

    return {
        "kuc": kuc.astype(bf16),
        "wz": np.ascontiguousarray(wz).astype(bf16),
        "xpw": xpw.astype(bf16),
        "dtw": np.ascontiguousarray(dt_w).astype(bf16),
        "ow": ow.astype(bf16),
        "emb": np.ascontiguousarray(emb_w).astype(bf16),
        "headw": np.ascontiguousarray(head_w).astype(bf16),
        "dtb": dtb2,
        "cb": cb2,
        "aneg": aneg,
        "dpar": dp2,
        "embb": np.asarray(emb_b).reshape(128, 1).astype(f32),
        "headb": np.asarray(head_b).reshape(1, 1).astype(f32),
    }


def kernel(features, emb_w, emb_b, in_proj_w, conv_w, conv_b, x_proj_w,
           dt_w, dt_b, A_log, D, out_proj_w, head_w, head_b,
           _tc_len=1024, _trace=False):
    from concourse.bass_utils import run_bass_kernel_spmd
    import ml_dtypes

    args = [np.asarray(a) for a in (
        features, emb_w, emb_b, in_proj_w, conv_w, conv_b, x_proj_w,
        dt_w, dt_b, A_log, D, out_proj_w, head_w, head_b)]
    features = args[0]
    b, s, _ = features.shape
    assert b == NCORES
    nc = _get_nc(s, _tc_len)
    common = prep_inputs(*args)
    in_maps = []
    for i in range(NCORES):
        m = dict(common)
        m["featT"] = np.ascontiguousarray(
            features[i].T).astype(ml_dtypes.bfloat16)
        in_maps.append(m)
    res = run_bass_kernel_spmd(nc, in_maps, core_ids=list(range(NCORES)),
                               trace=_trace)
    out = np.stack([r["out"].reshape(s, 1) for r in res.results])
    kernel.last_result = res
    return out.astype(np.float32)


# revision 7
# speedup vs baseline: 1.1838x; 1.1838x over previous
"""EventDenoisingMamba Trainium2 kernel (v2).

Data-parallel over batch: 8 batch elements -> 8 NeuronCores, one full
sequence (S=8192) per core. On chip: channels on partitions (2 blocks of
128 for d_inner=256, db-major along the free dim), time on the free dim.

v2 changes vs v1:
  - x activations live in DRAM ping-ping buffers (one per layer); each
    chunk loads a [128, L+3] input tile and stores a [128, L] output tile.
  - B/C broadcasts: xd rows bounce through DRAM once per chunk, then come
    back as FOUR grouped broadcast reads per side ([128, 4, L], 8KB per
    partition descriptors) instead of 32 per-row 2KB-descriptor DMAs.
  - Scan carries: the last h column of each (state, db) is copied by the
    Scalar engine into a small [128, 32] f32 carry tile (was: 512 strided
    DVE CASTs at 1.35us each); next chunk's scans use it as `initial`.
  - Elementwise mults/adds split DVE/GPSIMD by a tunable pattern.
  - ACT ops emitted silu-block-first then exp/ln-block to minimise
    activation-table switches.
"""

import numpy as np

import concourse.bass as bass
import concourse.tile as tile
from concourse import bacc, mybir

F32 = mybir.dt.float32
BF16 = mybir.dt.bfloat16
AF = mybir.ActivationFunctionType
OP = mybir.AluOpType

S = 8192
DM = 128      # d_model
DI = 256      # d_inner
NST = 16      # d_state
DC = 4        # d_conv
RK = 8        # dt_rank
NL = 4        # layers
NCORES = 8

# engine split tunables: which states' ops go to GPSIMD.
# GPSIMD runs ~5.9us per [128,2048] TT (vs 1.2us DVE) and its SBUF-port
# traffic slows concurrent DVE ops, so keep its share modest.
DBU_GPS = {3, 7, 11, 15}
TM_GPS = {1, 5, 9, 13}
ADD_GPS = {6, 14}


class Ctx:
    pass


def _bcast_view(t, r, L):
    """[128, 2, L] stride-0-partition-dup view of group tile row r."""
    return bass.AP(tensor=t.tensor, offset=t.offset + r * L,
                   ap=[list(t.ap[0]), [0, 2], [1, L]])


def _load_weights(c, nc, drams):
    wp = c.wp
    (kuc, wz, xpw, dtw, ow, emb, headw, dtb, cb, aneg, dpar, embb,
     headb) = drams
    c.w_kuc, c.w_wz, c.w_xpw, c.w_dtw, c.w_ow = [], [], [], [], []
    c.w_dtb, c.w_cb, c.w_a, c.w_d = [], [], [], []
    for l in range(NL):
        for lst, dram, shape, dt in [
            (c.w_kuc, kuc, [128, DC * DI], BF16),
            (c.w_wz, wz, [128, DI], BF16),
            (c.w_xpw, xpw, [128, 80], BF16),
            (c.w_dtw, dtw, [RK, DI], BF16),
            (c.w_ow, ow, [128, 256], BF16),
            (c.w_dtb, dtb, [128, 2], F32),
            (c.w_cb, cb, [128, 2], F32),
            (c.w_a, aneg, [128, 2 * NST], F32),
            (c.w_d, dpar, [128, 2], F32),
        ]:
            t = wp.tile(shape, dt, tag=f"w{len(lst)}_{id(lst) % 997}",
                        name=f"w{len(lst)}_{id(lst) % 997}")
            nc.sync.dma_start(t, dram[l])
            lst.append(t)
    c.w_emb = wp.tile([11, DM], BF16, tag="emb", name="emb")
    nc.sync.dma_start(c.w_emb, emb[:])
    c.w_headw = wp.tile([DM, 1], BF16, tag="headw", name="headw")
    nc.sync.dma_start(c.w_headw, headw[:])
    c.w_embb = wp.tile([128, 1], F32, tag="embb", name="embb")
    nc.sync.dma_start(c.w_embb, embb[:])
    c.w_headb = wp.tile([1, 1], F32, tag="headb", name="headb")
    nc.sync.dma_start(c.w_headb, headb[:])


def _embed(c, nc, featT, xbuf):
    L = c.L
    for blk in range(c.s // L):
        ps = c.pp.tile([128, L], F32, tag="mm", name="mm")
        for h2 in range(L // 512):
            col = blk * L + h2 * 512
            nc.tensor.matmul(
                ps[:, h2 * 512:(h2 + 1) * 512],
                lhsT=c.w_emb, rhs=c.w_feat[:, col:col + 512],
                start=True, stop=True)
        xo = c.xop.tile([128, L], BF16, tag="xo", name="xo")
        nc.scalar.activation(xo, ps, AF.Identity, bias=c.w_embb[:, 0:1])
        nc.sync.dma_start(xbuf[0][:, 3 + blk * L: 3 + (blk + 1) * L], xo)


def _chunk(c, nc, l, ci, xbuf, bcd):
    L = c.L
    t0 = ci * L

    xt = c.xtp.tile([128, L + 3], BF16, tag="xt", name="xt")
    nc.sync.dma_start(xt, xbuf[l][:, t0: t0 + L + 3])

    u = c.up.tile([128, 2 * L], BF16, tag="u", name="u")
    z = c.zp.tile([128, 2 * L], BF16, tag="z", name="z")
    de = c.dep.tile([128, 2 * L], BF16, tag="de", name="de")
    du = c.dup.tile([128, 2 * L], BF16, tag="du", name="du")

    # ---- u path (conv folded into 4 accumulated matmuls) + silu
    from concourse.tile_rust import add_dep_helper
    for db in range(2):
        ps = c.pp.tile([128, L], F32, tag="mm", name="mm")
        for h2 in range(L // 512):
            for k in range(DC):
                nc.tensor.matmul(
                    ps[:, h2 * 512:(h2 + 1) * 512],
                    lhsT=c.w_kuc[l][:, k * DI + db * 128:k * DI + db * 128 + 128],
                    rhs=xt[:, h2 * 512 + k: h2 * 512 + k + 512],
                    start=(k == 0), stop=(k == DC - 1))
        si = nc.scalar.activation(u[:, db * L:(db + 1) * L], ps, AF.Silu,
                                  bias=c.w_cb[l][:, db:db + 1])
        # keep ACT ops grouped by table-set: this chunk's silu block runs
        # after the previous chunk's last natural_log_exp op
        if db == 0 and getattr(c, "last_exp_inst", None) is not None:
            add_dep_helper(si.ins, c.last_exp_inst, sync=False,
                           reason="act table-set grouping")
    # ---- z path + silu
    for db in range(2):
        ps = c.pp.tile([128, L], F32, tag="mm", name="mm")
        for h2 in range(L // 512):
            nc.tensor.matmul(
                ps[:, h2 * 512:(h2 + 1) * 512],
                lhsT=c.w_wz[l][:, db * 128:db * 128 + 128],
                rhs=xt[:, 3 + h2 * 512: 3 + h2 * 512 + 512],
                start=True, stop=True)
        nc.scalar.activation(z[:, db * L:(db + 1) * L], ps, AF.Silu)

    # ---- x_dbl = u @ xp_w -> [40, L]
    xd = c.xdp.tile([40, L], BF16, tag="xd", name="xd")
    ps = c.pp.tile([128, L], F32, tag="mm", name="mm")
    for h2 in range(L // 512):
        for ct in range(2):
            nc.tensor.matmul(
                ps[0:40, h2 * 512:(h2 + 1) * 512],
                lhsT=c.w_xpw[l][:, ct * 40:ct * 40 + 40],
                rhs=u[:, ct * L + h2 * 512: ct * L + h2 * 512 + 512],
                start=(ct == 0), stop=(ct == 1))
    nc.scalar.activation(xd, ps[0:40, :], AF.Copy)

    # ---- B/C rows to DRAM, back as 4-row grouped partition-broadcasts
    slot = (l * c.nch + ci) % 4
    nc.gpsimd.dma_start(bcd[slot], xd[RK:RK + 2 * NST, :])
    bgrp, cgrp = [], []
    for g in range(4):
        bb = c.bbp.tile([128, 4, L], BF16, tag="bb", name="bb")
        grp = bcd[slot][4 * g:4 * g + 4, :]
        src = bass.AP(tensor=grp.tensor, offset=grp.offset,
                      ap=[[0, 128]] + [list(x) for x in grp.ap])
        (nc.sync if g % 2 == 0 else nc.scalar).dma_start(bb, src)
        bgrp.append(bb)
    for g in range(4):
        cc = c.ccp.tile([128, 4, L], BF16, tag="cc", name="cc")
        grp = bcd[slot][NST + 4 * g:NST + 4 * g + 4, :]
        src = bass.AP(tensor=grp.tensor, offset=grp.offset,
                      ap=[[0, 128]] + [list(x) for x in grp.ap])
        (nc.scalar if g % 2 == 0 else nc.sync).dma_start(cc, src)
        cgrp.append(cc)

    # ---- delta = softplus(dt @ dtw + dtb), per db
    # softplus(x) = relu(x) + ln(1 + exp(-|x|))
    for db in range(2):
        ps = c.pp.tile([128, L], F32, tag="mm", name="mm")
        for h2 in range(L // 512):
            nc.tensor.matmul(
                ps[:, h2 * 512:(h2 + 1) * 512],
                lhsT=c.w_dtw[l][:, db * 128:db * 128 + 128],
                rhs=xd[0:RK, h2 * 512:h2 * 512 + 512],
                start=True, stop=True)
        ab = c.abp.tile([128, L], F32, tag="ab", name="ab")
        nc.scalar.activation(ab, ps, AF.Abs, bias=c.w_dtb[l][:, db:db + 1])
        nc.scalar.activation(de[:, db * L:(db + 1) * L], ps, AF.Relu,
                             bias=c.w_dtb[l][:, db:db + 1])
        ab2 = c.ab2p.tile([128, L], BF16, tag="ab2", name="ab2")
        nc.scalar.activation(ab2, ab, AF.Exp, scale=-1.0)
        nc.scalar.activation(ab2, ab2, AF.Ln, bias=1.0)
        nc.vector.tensor_tensor(de[:, db * L:(db + 1) * L],
                                de[:, db * L:(db + 1) * L], ab2, OP.add)

    nc.vector.tensor_tensor(du, de, u, OP.mult)

    # ---- selective scan over 16 states
    hc = c.hcp.tile([128, 2 * NST], F32, tag="hc", name="hc")
    acc = [None, None]
    de3 = de.rearrange("p (b t) -> p b t", b=2)
    du3 = du.rearrange("p (b t) -> p b t", b=2)
    for n in range(NST):
        g, r = n // 4, n % 4
        # da = exp(delta * A_n) per db (f32 for scan precision)
        das = []
        for db in range(2):
            da = c.dap.tile([128, L], F32, tag="da", name="da")
            ei = nc.scalar.activation(
                da, de3[:, db, :], AF.Exp,
                scale=c.w_a[l][:, db * NST + n:db * NST + n + 1])
            das.append(da)
        c.last_exp_inst = ei.ins
        # dbu = du * B_n (B broadcast across partitions, dup over db)
        dbu = c.dbup.tile([128, 2 * L], BF16, tag="dbu", name="dbu")
        deng = nc.gpsimd if n in DBU_GPS else nc.vector
        deng.tensor_tensor(dbu.rearrange("p (b t) -> p b t", b=2), du3,
                           _bcast_view(bgrp[g], r, L), OP.mult)
        # scan: h = da * h_prev + dbu, chained via carry tile
        h = c.hp.tile([128, 2 * L], BF16, tag="h", name="h")
        for db in range(2):
            init = 0.0 if ci == 0 else c.hc_prev[:, 2 * n + db:2 * n + db + 1]
            nc.vector.tensor_tensor_scan(
                h[:, db * L:(db + 1) * L], das[db],
                dbu[:, db * L:(db + 1) * L],
                initial=init, op0=OP.mult, op1=OP.add)
        # carry out last column of both db segments (Scalar engine copy)
        hsrc = bass.AP(tensor=h.tensor, offset=h.offset + L - 1,
                       ap=[list(h.ap[0]), [L, 2]])
        nc.scalar.activation(hc[:, 2 * n:2 * n + 2], hsrc, AF.Copy)
        # tm = h * C_n ; accumulate into acc[n % 2]
        teng = nc.gpsimd if n in TM_GPS else nc.vector
        if n < 2:
            a = c.accp.tile([128, 2 * L], BF16, tag=f"acc{n}", name="acc")
            teng.tensor_tensor(a.rearrange("p (b t) -> p b t", b=2),
                               h.rearrange("p (b t) -> p b t", b=2),
                               _bcast_view(cgrp[g], r, L), OP.mult)
            acc[n] = a
        else:
            tm = c.tmp.tile([128, 2 * L], BF16, tag="tm", name="tm")
            teng.tensor_tensor(tm.rearrange("p (b t) -> p b t", b=2),
                               h.rearrange("p (b t) -> p b t", b=2),
                               _bcast_view(cgrp[g], r, L), OP.mult)
            aeng = nc.gpsimd if n in ADD_GPS else nc.vector
            aeng.tensor_tensor(acc[n % 2], acc[n % 2], tm, OP.add)
    c.hc_prev = hc

    # ---- y = acc0 + acc1 + u*D, gated by silu(z)
    y = acc[0]
    nc.vector.tensor_tensor(y, y, acc[1], OP.add)
    for db in range(2):
        sl = slice(db * L, (db + 1) * L)
        nc.vector.scalar_tensor_tensor(
            y[:, sl], u[:, sl], c.w_d[l][:, db:db + 1], y[:, sl],
            OP.mult, OP.add)
    nc.vector.tensor_tensor(y, y, z, OP.mult)

    # ---- out_proj -> next layer's x buffer
    ps = c.pp.tile([128, L], F32, tag="mm", name="mm")
    for h2 in range(L // 512):
        for ct in range(2):
            nc.tensor.matmul(
                ps[:, h2 * 512:(h2 + 1) * 512],
                lhsT=c.w_ow[l][:, ct * 128:ct * 128 + 128],
                rhs=y[:, ct * L + h2 * 512: ct * L + h2 * 512 + 512],
                start=(ct == 0), stop=(ct == 1))
    xo = c.xop.tile([128, L], BF16, tag="xo", name="xo")
    nc.scalar.activation(xo, ps, AF.Copy)
    nc.scalar.dma_start(xbuf[l + 1][:, 3 + t0: 3 + t0 + L], xo)


def _head(c, nc, xbuf, out):
    L = c.L
    for blk in range(c.s // L):
        xt = c.xtp.tile([128, L + 3], BF16, tag="xt", name="xt")
        nc.sync.dma_start(xt, xbuf[NL][:, blk * L: blk * L + L + 3])
        ps = c.pp.tile([128, L], F32, tag="mm", name="mm")
        for h2 in range(L // 512):
            nc.tensor.matmul(
                ps[0:1, h2 * 512:(h2 + 1) * 512],
                lhsT=c.w_headw, rhs=xt[:, 3 + h2 * 512: 3 + h2 * 512 + 512],
                start=True, stop=True)
        ot = c.abp.tile([128, L], F32, tag="ab", name="ot")
        nc.scalar.activation(ot[0:1, :], ps[0:1, :], AF.Sigmoid,
                             bias=c.w_headb[0:1, 0:1])
        nc.sync.dma_start(out[0:1, blk * L:(blk + 1) * L], ot[0:1, :])


def build(s=S, tc_len=1024, nloops=1):
    nc = bacc.Bacc("TRN2", target_bir_lowering=False, debug=False,
                   num_devices=NCORES)
    drams = (
        nc.declare_dram_parameter("kuc", [NL, 128, DC * DI], BF16, False),
        nc.declare_dram_parameter("wz", [NL, 128, DI], BF16, False),
        nc.declare_dram_parameter("xpw", [NL, 128, 80], BF16, False),
        nc.declare_dram_parameter("dtw", [NL, RK, DI], BF16, False),
        nc.declare_dram_parameter("ow", [NL, 128, 256], BF16, False),
        nc.declare_dram_parameter("emb", [11, DM], BF16, False),
        nc.declare_dram_parameter("headw", [DM, 1], BF16, False),
        nc.declare_dram_parameter("dtb", [NL, 128, 2], F32, False),
        nc.declare_dram_parameter("cb", [NL, 128, 2], F32, False),
        nc.declare_dram_parameter("aneg", [NL, 128, 2 * NST], F32, False),
        nc.declare_dram_parameter("dpar", [NL, 128, 2], F32, False),
        nc.declare_dram_parameter("embb", [128, 1], F32, False),
        nc.declare_dram_parameter("headb", [1, 1], F32, False),
    )
    featT = nc.declare_dram_parameter("featT", [11, s], BF16, False)
    out = nc.declare_dram_parameter("out", [1, s], F32, True)
    bcd = nc.dram_tensor("bcd", [4, 2 * NST, tc_len], BF16)
    xbuf = [nc.dram_tensor(f"xb{i}", [128, 3 + s], BF16)
            for i in range(NL + 1)]

    c = Ctx()
    c.s = s
    c.L = tc_len
    c.nch = s // tc_len

    with tile.TileContext(nc) as tcx:
        with (
            tcx.tile_pool(name="w", bufs=1) as wp,
            tcx.tile_pool(name="psP", bufs=3, space="PSUM") as pp,
            tcx.tile_pool(name="xo", bufs=3) as xop,
            tcx.tile_pool(name="ab", bufs=2) as abp,
        ):
            c.wp, c.pp, c.xop, c.abp = wp, pp, xop, abp
            _load_weights(c, nc, drams)

            zt = wp.tile([128, 3], BF16, tag="zpad", name="zpad")
            nc.vector.memset(zt, 0.0)
            for i in range(NL + 1):
                nc.sync.dma_start(xbuf[i][:, 0:3], zt)

            with tcx.tile_pool(name="feat", bufs=1) as fp:
                c.w_feat = fp.tile([11, s], BF16, tag="featT", name="featT")
                nc.sync.dma_start(c.w_feat, featT[:])
                _embed(c, nc, featT, xbuf)

            from contextlib import ExitStack
            with ExitStack() as es:
                pools = {
                    "xtp": ("xt", 3), "up": ("u", 2), "zp": ("z", 2),
                    "dep": ("de", 2), "dup": ("du", 2), "xdp": ("xd", 2),
                    "bbp": ("bb", 2), "ccp": ("cc", 2), "dap": ("da", 4),
                    "dbup": ("dbu", 4), "hp": ("h", 6), "hcp": ("hcp", 2),
                    "tmp": ("tm", 4), "accp": ("accp", 2),
                    "ab2p": ("ab2", 2),
                }
                for attr, (nm, bufs) in pools.items():
                    setattr(c, attr,
                            es.enter_context(tcx.tile_pool(name=nm, bufs=bufs)))

                for rep in range(nloops):
                    for l in range(NL):
                        for ci in range(c.nch):
                            _chunk(c, nc, l, ci, xbuf, bcd)
                _head(c, nc, xbuf, out)

    nc.compile()
    return nc


_CACHE = {}


def _get_nc(s, tc_len, nloops=1):
    key = (s, tc_len, nloops)
    if key not in _CACHE:
        _CACHE[key] = build(s, tc_len, nloops)
    return _CACHE[key]


def prep_inputs(features, emb_w, emb_b, in_proj_w, conv_w, conv_b, x_proj_w,
                dt_w, dt_b, A_log, D, out_proj_w, head_w, head_b):
    """Host-side weight preprocessing shared by all cores."""
    import ml_dtypes
    f32 = np.float32
    bf16 = ml_dtypes.bfloat16

    nl = in_proj_w.shape[0]
    kuc = np.zeros((nl, 128, DC * DI), dtype=f32)
    for l in range(nl):
        wu = in_proj_w[l][:, :DI]                      # [128, 256]
        for k in range(DC):
            kuc[l][:, k * DI:(k + 1) * DI] = wu * conv_w[l][:, k][None, :]
    wz = in_proj_w[:, :, DI:]                          # [NL, 128, 256]
    xpw = np.zeros((nl, 128, 80), dtype=f32)
    ow = np.zeros((nl, 128, 256), dtype=f32)
    aneg = np.zeros((nl, 128, 2 * NST), dtype=f32)
    dtb2 = np.zeros((nl, 128, 2), dtype=f32)
    cb2 = np.zeros((nl, 128, 2), dtype=f32)
    dp2 = np.zeros((nl, 128, 2), dtype=f32)
    for l in range(nl):
        for ct in range(2):
            xpw[l][:, ct * 40:(ct + 1) * 40] = \
                x_proj_w[l][ct * 128:(ct + 1) * 128, :]
            ow[l][:, ct * 128:(ct + 1) * 128] = \
                out_proj_w[l][ct * 128:(ct + 1) * 128, :]
            aneg[l][:, ct * NST:(ct + 1) * NST] = \
                -np.exp(A_log[l][ct * 128:(ct + 1) * 128, :])
            dtb2[l][:, ct] = dt_b[l][ct * 128:(ct + 1) * 128]
            cb2[l][:, ct] = conv_b[l][ct * 128:(ct + 1) * 128]
            dp2[l][:, ct] = D[l][ct * 128:(ct + 1) * 128]

    return {
        "kuc": kuc.astype(bf16),
        "wz": np.ascontiguousarray(wz).astype(bf16),
        "xpw": xpw.astype(bf16),
        "dtw": np.ascontiguousarray(dt_w).astype(bf16),
        "ow": ow.astype(bf16),
        "emb": np.ascontiguousarray(emb_w).astype(bf16),
        "headw": np.ascontiguousarray(head_w).astype(bf16),
        "dtb": dtb2,
        "cb": cb2,
        "aneg": aneg,
        "dpar": dp2,
        "embb": np.asarray(emb_b).reshape(128, 1).astype(f32),
        "headb": np.asarray(head_b).reshape(1, 1).astype(f32),
    }


def kernel(features, emb_w, emb_b, in_proj_w, conv_w, conv_b, x_proj_w,
           dt_w, dt_b, A_log, D, out_proj_w, head_w, head_b,
           _tc_len=1024, _trace=False):
    from concourse.bass_utils import run_bass_kernel_spmd
    import ml_dtypes

    args = [np.asarray(a) for a in (
        features, emb_w, emb_b, in_proj_w, conv_w, conv_b, x_proj_w,
        dt_w, dt_b, A_log, D, out_proj_w, head_w, head_b)]
    features = args[0]
    b, s, _ = features.shape
    assert b == NCORES
    nc = _get_nc(s, _tc_len)
    common = prep_inputs(*args)
    in_maps = []
    for i in range(NCORES):
        m = dict(common)
        m["featT"] = np.ascontiguousarray(
            features[i].T).astype(ml_dtypes.bfloat16)
        in_maps.append(m)
    res = run_bass_kernel_spmd(nc, in_maps, core_ids=list(range(NCORES)),
                               trace=_trace)
    out = np.stack([r["out"].reshape(s, 1) for r in res.results])
    kernel.last_result = res
    return out.astype(np.float32)


# revision 11
# speedup vs baseline: 1.3556x; 1.1451x over previous
"""EventDenoisingMamba Trainium2 kernel (v2).

Data-parallel over batch: 8 batch elements -> 8 NeuronCores, one full
sequence (S=8192) per core. On chip: channels on partitions (2 blocks of
128 for d_inner=256, db-major along the free dim), time on the free dim.

v2 changes vs v1:
  - x activations live in DRAM ping-ping buffers (one per layer); each
    chunk loads a [128, L+3] input tile and stores a [128, L] output tile.
  - B/C broadcasts: xd rows bounce through DRAM once per chunk, then come
    back as FOUR grouped broadcast reads per side ([128, 4, L], 8KB per
    partition descriptors) instead of 32 per-row 2KB-descriptor DMAs.
  - Scan carries: the last h column of each (state, db) is copied by the
    Scalar engine into a small [128, 32] f32 carry tile (was: 512 strided
    DVE CASTs at 1.35us each); next chunk's scans use it as `initial`.
  - Elementwise mults/adds split DVE/GPSIMD by a tunable pattern.
  - ACT ops emitted silu-block-first then exp/ln-block to minimise
    activation-table switches.
"""

import numpy as np

import concourse.bass as bass
import concourse.tile as tile
from concourse import bacc, mybir

F32 = mybir.dt.float32
BF16 = mybir.dt.bfloat16
AF = mybir.ActivationFunctionType
OP = mybir.AluOpType

S = 8192
DM = 128      # d_model
DI = 256      # d_inner
NST = 16      # d_state
DC = 4        # d_conv
RK = 8        # dt_rank
NL = 4        # layers
NCORES = 8

# engine split tunables: which states' ops go to GPSIMD.
# GPSIMD runs ~5.9us per [128,2048] TT (vs 1.2us DVE) and its SBUF-port
# traffic slows concurrent DVE ops, so keep it off the per-state critical
# chain (dbu -> scan -> tm): it only takes accumulator adds + frontend ops.
DBU_GPS = set()
TM_GPS = set()
ADD_GPS = {2, 4, 6, 8, 10, 12, 14}


def _patch_act_tables():
    """Steer the greedy ACT-table-set chooser so softplus' exp/ln and the
    per-state da exps all resolve to natural_log_exp_and_others (one table
    set) instead of ping-ponging exp_and_others <-> natural_log.  Only the
    chooser's view is patched; emitted act_func_set_ids still index real
    sets that contain the functions, so runtime tables are correct."""
    from concourse import bacc as _bacc
    if getattr(_bacc, "_act_tables_patched", False):
        return
    _orig = _bacc.get_activation_tables

    def _patched(module_arch):
        t = _orig(module_arch)
        out = {}
        for name, funcs in t.items():
            f = set(funcs)
            if name == "exp_and_others":
                f.discard(AF.Exp)
            if name == "natural_log":
                f.discard(AF.Ln)
            out[name] = f
        return out

    _bacc.get_activation_tables = _patched
    _bacc._act_tables_patched = True


class Ctx:
    pass


def _bcast_view(t, r, L):
    """[128, 2, L] stride-0-partition-dup view of group tile row r."""
    return bass.AP(tensor=t.tensor, offset=t.offset + r * L,
                   ap=[list(t.ap[0]), [0, 2], [1, L]])


def _load_weights(c, nc, drams):
    wp = c.wp
    (kuc, wz, xpw, dtw, ow, emb, headw, dtb, cb, aneg, dpar, embb,
     headb) = drams
    c.w_kuc, c.w_wz, c.w_xpw, c.w_dtw, c.w_ow = [], [], [], [], []
    c.w_dtb, c.w_cb, c.w_a, c.w_d = [], [], [], []
    for l in range(NL):
        for lst, dram, shape, dt in [
            (c.w_kuc, kuc, [128, DC * DI], BF16),
            (c.w_wz, wz, [128, DI], BF16),
            (c.w_xpw, xpw, [128, 80], BF16),
            (c.w_dtw, dtw, [RK, DI], BF16),
            (c.w_ow, ow, [128, 256], BF16),
            (c.w_dtb, dtb, [128, 2], F32),
            (c.w_cb, cb, [128, 2], F32),
            (c.w_a, aneg, [128, 2 * NST], F32),
            (c.w_d, dpar, [128, 2], F32),
        ]:
            t = wp.tile(shape, dt, tag=f"w{len(lst)}_{id(lst) % 997}",
                        name=f"w{len(lst)}_{id(lst) % 997}")
            nc.sync.dma_start(t, dram[l])
            lst.append(t)
    c.w_emb = wp.tile([11, DM], BF16, tag="emb", name="emb")
    nc.sync.dma_start(c.w_emb, emb[:])
    c.w_headw = wp.tile([DM, 1], BF16, tag="headw", name="headw")
    nc.sync.dma_start(c.w_headw, headw[:])
    c.w_embb = wp.tile([128, 1], F32, tag="embb", name="embb")
    nc.sync.dma_start(c.w_embb, embb[:])
    c.w_headb = wp.tile([1, 1], F32, tag="headb", name="headb")
    nc.sync.dma_start(c.w_headb, headb[:])


def _embed(c, nc, featT, xbuf):
    L = c.L
    for blk in range(c.s // L):
        ps = c.pp.tile([128, L], F32, tag="mm", name="mm")
        for h2 in range(L // 512):
            col = blk * L + h2 * 512
            nc.tensor.matmul(
                ps[:, h2 * 512:(h2 + 1) * 512],
                lhsT=c.w_emb, rhs=c.w_feat[:, col:col + 512],
                start=True, stop=True)
        xo = c.xop.tile([128, L], BF16, tag="xo", name="xo")
        nc.scalar.activation(xo, ps, AF.Identity, bias=c.w_embb[:, 0:1])
        nc.sync.dma_start(xbuf[0][:, 3 + blk * L: 3 + (blk + 1) * L], xo)


def _front(c, nc, l, ci, xbuf, bcd):
    """Chunk frontend: projections, conv, silu, x_dbl, softplus, broadcasts.
    Emitted one chunk ahead of the state loop so its serial latency chain
    (and ACT table loads) hide under the previous chunk's scan work."""
    L = c.L
    t0 = ci * L
    fr = Ctx()

    xt = c.xtp.tile([128, L + 3], BF16, tag="xt", name="xt")
    nc.sync.dma_start(xt, xbuf[l][:, t0: t0 + L + 3])

    u = c.up.tile([128, 2 * L], BF16, tag="u", name="u")
    z = c.zp.tile([128, 2 * L], BF16, tag="z", name="z")
    de = c.dep.tile([128, 2 * L], BF16, tag="de", name="de")
    du = c.dup.tile([128, 2 * L], BF16, tag="du", name="du")

    # ---- u path (conv folded into 4 accumulated matmuls) + silu
    for db in range(2):
        ps = c.pp.tile([128, L], F32, tag="mm", name="mm")
        for h2 in range(L // 512):
            for k in range(DC):
                nc.tensor.matmul(
                    ps[:, h2 * 512:(h2 + 1) * 512],
                    lhsT=c.w_kuc[l][:, k * DI + db * 128:k * DI + db * 128 + 128],
                    rhs=xt[:, h2 * 512 + k: h2 * 512 + k + 512],
                    start=(k == 0), stop=(k == DC - 1))
        nc.scalar.activation(u[:, db * L:(db + 1) * L], ps, AF.Silu,
                             bias=c.w_cb[l][:, db:db + 1])
    # ---- z path + silu
    for db in range(2):
        ps = c.pp.tile([128, L], F32, tag="mm", name="mm")
        for h2 in range(L // 512):
            nc.tensor.matmul(
                ps[:, h2 * 512:(h2 + 1) * 512],
                lhsT=c.w_wz[l][:, db * 128:db * 128 + 128],
                rhs=xt[:, 3 + h2 * 512: 3 + h2 * 512 + 512],
                start=True, stop=True)
        nc.scalar.activation(z[:, db * L:(db + 1) * L], ps, AF.Silu)

    # ---- x_dbl = u @ xp_w -> [40, L]
    xd = c.xdp.tile([40, L], BF16, tag="xd", name="xd")
    ps = c.pp.tile([128, L], F32, tag="mm", name="mm")
    for h2 in range(L // 512):
        for ct in range(2):
            nc.tensor.matmul(
                ps[0:40, h2 * 512:(h2 + 1) * 512],
                lhsT=c.w_xpw[l][:, ct * 40:ct * 40 + 40],
                rhs=u[:, ct * L + h2 * 512: ct * L + h2 * 512 + 512],
                start=(ct == 0), stop=(ct == 1))
    nc.scalar.activation(xd, ps[0:40, :], AF.Copy)

    # ---- B/C rows to DRAM, back as 4-row grouped partition-broadcasts
    slot = (l * c.nch + ci) % 4
    nc.gpsimd.dma_start(bcd[slot], xd[RK:RK + 2 * NST, :])
    fr.bgrp, fr.cgrp = [], []
    for g in range(4):
        bb = c.bbp.tile([128, 4, L], BF16, tag="bb", name="bb")
        grp = bcd[slot][4 * g:4 * g + 4, :]
        src = bass.AP(tensor=grp.tensor, offset=grp.offset,
                      ap=[[0, 128]] + [list(x) for x in grp.ap])
        (nc.sync if g % 2 == 0 else nc.scalar).dma_start(bb, src)
        fr.bgrp.append(bb)
    for g in range(4):
        cc = c.ccp.tile([128, 4, L], BF16, tag="cc", name="cc")
        grp = bcd[slot][NST + 4 * g:NST + 4 * g + 4, :]
        src = bass.AP(tensor=grp.tensor, offset=grp.offset,
                      ap=[[0, 128]] + [list(x) for x in grp.ap])
        (nc.scalar if g % 2 == 0 else nc.sync).dma_start(cc, src)
        fr.cgrp.append(cc)

    # ---- delta = softplus(dt @ dtw + dtb), both db, set-batched ACT ops
    # softplus(x) = relu(x) + ln(1 + exp(-|x|))
    pss, abs_, ab2s = [], [], []
    for db in range(2):
        ps = c.pp.tile([128, L], F32, tag="mm", name="mm")
        for h2 in range(L // 512):
            nc.tensor.matmul(
                ps[:, h2 * 512:(h2 + 1) * 512],
                lhsT=c.w_dtw[l][:, db * 128:db * 128 + 128],
                rhs=xd[0:RK, h2 * 512:h2 * 512 + 512],
                start=True, stop=True)
        pss.append(ps)
    for db in range(2):
        ab = c.abp.tile([128, L], F32, tag="ab", name="ab")
        nc.scalar.activation(ab, pss[db], AF.Abs,
                             bias=c.w_dtb[l][:, db:db + 1])
        abs_.append(ab)
    for db in range(2):
        nc.scalar.activation(de[:, db * L:(db + 1) * L], pss[db], AF.Relu,
                             bias=c.w_dtb[l][:, db:db + 1])
    for db in range(2):
        ab2 = c.ab2p.tile([128, L], BF16, tag="ab2", name="ab2")
        nc.scalar.activation(ab2, abs_[db], AF.Exp, scale=-1.0)
        ab2s.append(ab2)
    for db in range(2):
        nc.scalar.activation(ab2s[db], ab2s[db], AF.Ln, bias=1.0)
    for db in range(2):
        nc.gpsimd.tensor_tensor(de[:, db * L:(db + 1) * L],
                                de[:, db * L:(db + 1) * L], ab2s[db], OP.add)

    nc.gpsimd.tensor_tensor(du, de, u, OP.mult)
    fr.u, fr.z, fr.de, fr.du = u, z, de, du
    return fr


def _states(c, nc, l, ci, fr, xbuf):
    L = c.L
    t0 = ci * L
    u, z, de, du = fr.u, fr.z, fr.de, fr.du

    # ---- selective scan over 16 states
    hc = c.hcp.tile([128, 2 * NST], F32, tag="hc", name="hc")
    acc = [None, None]
    de3 = de.rearrange("p (b t) -> p b t", b=2)
    du3 = du.rearrange("p (b t) -> p b t", b=2)
    for n in range(NST):
        g, r = n // 4, n % 4
        # da = exp(delta * A_n) per db (f32 for scan precision)
        das = []
        for db in range(2):
            da = c.dap.tile([128, L], F32, tag="da", name="da")
            nc.scalar.activation(
                da, de3[:, db, :], AF.Exp,
                scale=c.w_a[l][:, db * NST + n:db * NST + n + 1])
            das.append(da)
        # dbu = du * B_n (B broadcast across partitions, dup over db)
        dbu = c.dbup.tile([128, 2 * L], BF16, tag="dbu", name="dbu")
        deng = nc.gpsimd if n in DBU_GPS else nc.vector
        deng.tensor_tensor(dbu.rearrange("p (b t) -> p b t", b=2), du3,
                           _bcast_view(fr.bgrp[g], r, L), OP.mult)
        # scan: h = da * h_prev + dbu, chained via carry tile
        h = c.hp.tile([128, 2 * L], BF16, tag="h", name="h")
        for db in range(2):
            init = 0.0 if ci == 0 else c.hc_prev[:, 2 * n + db:2 * n + db + 1]
            nc.vector.tensor_tensor_scan(
                h[:, db * L:(db + 1) * L], das[db],
                dbu[:, db * L:(db + 1) * L],
                initial=init, op0=OP.mult, op1=OP.add)
        # carry out last column of both db segments (Scalar engine copy)
        hsrc = bass.AP(tensor=h.tensor, offset=h.offset + L - 1,
                       ap=[list(h.ap[0]), [L, 2]])
        nc.scalar.activation(hc[:, 2 * n:2 * n + 2], hsrc, AF.Copy)
        # tm = h * C_n ; accumulate into acc[n % 2]
        teng = nc.gpsimd if n in TM_GPS else nc.vector
        if n < 2:
            a = c.accp.tile([128, 2 * L], BF16, tag=f"acc{n}", name="acc")
            teng.tensor_tensor(a.rearrange("p (b t) -> p b t", b=2),
                               h.rearrange("p (b t) -> p b t", b=2),
                               _bcast_view(fr.cgrp[g], r, L), OP.mult)
            acc[n] = a
        else:
            tm = c.tmp.tile([128, 2 * L], BF16, tag="tm", name="tm")
            teng.tensor_tensor(tm.rearrange("p (b t) -> p b t", b=2),
                               h.rearrange("p (b t) -> p b t", b=2),
                               _bcast_view(fr.cgrp[g], r, L), OP.mult)
            aeng = nc.gpsimd if n in ADD_GPS else nc.vector
            aeng.tensor_tensor(acc[n % 2], acc[n % 2], tm, OP.add)
    c.hc_prev = hc

    # ---- y = acc0 + acc1 + u*D, gated by silu(z)
    y = acc[0]
    nc.vector.tensor_tensor(y, y, acc[1], OP.add)
    for db in range(2):
        sl = slice(db * L, (db + 1) * L)
        nc.vector.scalar_tensor_tensor(
            y[:, sl], u[:, sl], c.w_d[l][:, db:db + 1], y[:, sl],
            OP.mult, OP.add)
    nc.vector.tensor_tensor(y, y, z, OP.mult)

    # ---- out_proj -> next layer's x buffer
    ps = c.pp.tile([128, L], F32, tag="mm", name="mm")
    for h2 in range(L // 512):
        for ct in range(2):
            nc.tensor.matmul(
                ps[:, h2 * 512:(h2 + 1) * 512],
                lhsT=c.w_ow[l][:, ct * 128:ct * 128 + 128],
                rhs=y[:, ct * L + h2 * 512: ct * L + h2 * 512 + 512],
                start=(ct == 0), stop=(ct == 1))
    xo = c.xop.tile([128, L], BF16, tag="xo", name="xo")
    nc.scalar.activation(xo, ps, AF.Copy)
    nc.scalar.dma_start(xbuf[l + 1][:, 3 + t0: 3 + t0 + L], xo)


def _head(c, nc, xbuf, out):
    L = c.L
    for blk in range(c.s // L):
        xt = c.xtp.tile([128, L + 3], BF16, tag="xt", name="xt")
        nc.sync.dma_start(xt, xbuf[NL][:, blk * L: blk * L + L + 3])
        ps = c.pp.tile([128, L], F32, tag="mm", name="mm")
        for h2 in range(L // 512):
            nc.tensor.matmul(
                ps[0:1, h2 * 512:(h2 + 1) * 512],
                lhsT=c.w_headw, rhs=xt[:, 3 + h2 * 512: 3 + h2 * 512 + 512],
                start=True, stop=True)
        ot = c.abp.tile([128, L], F32, tag="ab", name="ot")
        nc.scalar.activation(ot[0:1, :], ps[0:1, :], AF.Sigmoid,
                             bias=c.w_headb[0:1, 0:1])
        nc.sync.dma_start(out[0:1, blk * L:(blk + 1) * L], ot[0:1, :])


def build(s=S, tc_len=1024, nloops=1):
    _patch_act_tables()
    nc = bacc.Bacc("TRN2", target_bir_lowering=False, debug=False,
                   num_devices=NCORES)
    drams = (
        nc.declare_dram_parameter("kuc", [NL, 128, DC * DI], BF16, False),
        nc.declare_dram_parameter("wz", [NL, 128, DI], BF16, False),
        nc.declare_dram_parameter("xpw", [NL, 128, 80], BF16, False),
        nc.declare_dram_parameter("dtw", [NL, RK, DI], BF16, False),
        nc.declare_dram_parameter("ow", [NL, 128, 256], BF16, False),
        nc.declare_dram_parameter("emb", [11, DM], BF16, False),
        nc.declare_dram_parameter("headw", [DM, 1], BF16, False),
        nc.declare_dram_parameter("dtb", [NL, 128, 2], F32, False),
        nc.declare_dram_parameter("cb", [NL, 128, 2], F32, False),
        nc.declare_dram_parameter("aneg", [NL, 128, 2 * NST], F32, False),
        nc.declare_dram_parameter("dpar", [NL, 128, 2], F32, False),
        nc.declare_dram_parameter("embb", [128, 1], F32, False),
        nc.declare_dram_parameter("headb", [1, 1], F32, False),
    )
    featT = nc.declare_dram_parameter("featT", [11, s], BF16, False)
    out = nc.declare_dram_parameter("out", [1, s], F32, True)
    bcd = nc.dram_tensor("bcd", [4, 2 * NST, tc_len], BF16)
    xbuf = [nc.dram_tensor(f"xb{i}", [128, 3 + s], BF16)
            for i in range(NL + 1)]

    c = Ctx()
    c.s = s
    c.L = tc_len
    c.nch = s // tc_len

    with tile.TileContext(nc) as tcx:
        with (
            tcx.tile_pool(name="w", bufs=1) as wp,
            tcx.tile_pool(name="psP", bufs=3, space="PSUM") as pp,
            tcx.tile_pool(name="xo", bufs=3) as xop,
            tcx.tile_pool(name="ab", bufs=2) as abp,
        ):
            c.wp, c.pp, c.xop, c.abp = wp, pp, xop, abp
            _load_weights(c, nc, drams)

            zt = wp.tile([128, 3], BF16, tag="zpad", name="zpad")
            nc.vector.memset(zt, 0.0)
            for i in range(NL + 1):
                nc.sync.dma_start(xbuf[i][:, 0:3], zt)

            with tcx.tile_pool(name="feat", bufs=1) as fp:
                c.w_feat = fp.tile([11, s], BF16, tag="featT", name="featT")
                nc.sync.dma_start(c.w_feat, featT[:])
                _embed(c, nc, featT, xbuf)

            from contextlib import ExitStack
            with ExitStack() as es:
                pools = {
                    "xtp": ("xt", 3), "up": ("u", 2), "zp": ("z", 2),
                    "dep": ("de", 2), "dup": ("du", 2), "xdp": ("xd", 2),
                    "bbp": ("bb", 2), "ccp": ("cc", 2), "dap": ("da", 4),
                    "dbup": ("dbu", 4), "hp": ("h", 6), "hcp": ("hcp", 2),
                    "tmp": ("tm", 4), "accp": ("accp", 2),
                    "ab2p": ("ab2", 2),
                }
                for attr, (nm, bufs) in pools.items():
                    setattr(c, attr,
                            es.enter_context(tcx.tile_pool(name=nm, bufs=bufs)))

                chunks = [(l, ci) for rep in range(nloops)
                          for l in range(NL) for ci in range(c.nch)]
                fr = _front(c, nc, chunks[0][0], chunks[0][1], xbuf, bcd)
                for idx, (l, ci) in enumerate(chunks):
                    nxt = chunks[idx + 1] if idx + 1 < len(chunks) else None
                    fr_next = (_front(c, nc, nxt[0], nxt[1], xbuf, bcd)
                               if nxt else None)
                    _states(c, nc, l, ci, fr, xbuf)
                    fr = fr_next
                _head(c, nc, xbuf, out)

    nc.compile()
    return nc


_CACHE = {}


def _get_nc(s, tc_len, nloops=1):
    key = (s, tc_len, nloops)
    if key not in _CACHE:
        _CACHE[key] = build(s, tc_len, nloops)
    return _CACHE[key]


def prep_inputs(features, emb_w, emb_b, in_proj_w, conv_w, conv_b, x_proj_w,
                dt_w, dt_b, A_log, D, out_proj_w, head_w, head_b):
    """Host-side weight preprocessing shared by all cores."""
    import ml_dtypes
    f32 = np.float32
    bf16 = ml_dtypes.bfloat16

    nl = in_proj_w.shape[0]
    kuc = np.zeros((nl, 128, DC * DI), dtype=f32)
    for l in range(nl):
        wu = in_proj_w[l][:, :DI]                      # [128, 256]
        for k in range(DC):
            kuc[l][:, k * DI:(k + 1) * DI] = wu * conv_w[l][:, k][None, :]
    wz = in_proj_w[:, :, DI:]                          # [NL, 128, 256]
    xpw = np.zeros((nl, 128, 80), dtype=f32)
    ow = np.zeros((nl, 128, 256), dtype=f32)
    aneg = np.zeros((nl, 128, 2 * NST), dtype=f32)
    dtb2 = np.zeros((nl, 128, 2), dtype=f32)
    cb2 = np.zeros((nl, 128, 2), dtype=f32)
    dp2 = np.zeros((nl, 128, 2), dtype=f32)
    for l in range(nl):
        for ct in range(2):
            xpw[l][:, ct * 40:(ct + 1) * 40] = \
                x_proj_w[l][ct * 128:(ct + 1) * 128, :]
            ow[l][:, ct * 128:(ct + 1) * 128] = \
                out_proj_w[l][ct * 128:(ct + 1) * 128, :]
            aneg[l][:, ct * NST:(ct + 1) * NST] = \
                -np.exp(A_log[l][ct * 128:(ct + 1) * 128, :])
            dtb2[l][:, ct] = dt_b[l][ct * 128:(ct + 1) * 128]
            cb2[l][:, ct] = conv_b[l][ct * 128:(ct + 1) * 128]
            dp2[l][:, ct] = D[l][ct * 128:(ct + 1) * 128]

    return {
        "kuc": kuc.astype(bf16),
        "wz": np.ascontiguousarray(wz).astype(bf16),
        "xpw": xpw.astype(bf16),
        "dtw": np.ascontiguousarray(dt_w).astype(bf16),
        "ow": ow.astype(bf16),
        "emb": np.ascontiguousarray(emb_w).astype(bf16),
        "headw": np.ascontiguousarray(head_w).astype(bf16),
        "dtb": dtb2,
        "cb": cb2,
        "aneg": aneg,
        "dpar": dp2,
        "embb": np.asarray(emb_b).reshape(128, 1).astype(f32),
        "headb": np.asarray(head_b).reshape(1, 1).astype(f32),
    }


def kernel(features, emb_w, emb_b, in_proj_w, conv_w, conv_b, x_proj_w,
           dt_w, dt_b, A_log, D, out_proj_w, head_w, head_b,
           _tc_len=1024, _trace=False):
    from concourse.bass_utils import run_bass_kernel_spmd
    import ml_dtypes

    args = [np.asarray(a) for a in (
        features, emb_w, emb_b, in_proj_w, conv_w, conv_b, x_proj_w,
        dt_w, dt_b, A_log, D, out_proj_w, head_w, head_b)]
    features = args[0]
    b, s, _ = features.shape
    assert b == NCORES
    nc = _get_nc(s, _tc_len)
    common = prep_inputs(*args)
    in_maps = []
    for i in range(NCORES):
        m = dict(common)
        m["featT"] = np.ascontiguousarray(
            features[i].T).astype(ml_dtypes.bfloat16)
        in_maps.append(m)
    res = run_bass_kernel_spmd(nc, in_maps, core_ids=list(range(NCORES)),
                               trace=_trace)
    out = np.stack([r["out"].reshape(s, 1) for r in res.results])
    kernel.last_result = res
    return out.astype(np.float32)


# revision 12
# speedup vs baseline: 1.6319x; 1.2039x over previous
"""EventDenoisingMamba Trainium2 kernel (v2).

Data-parallel over batch: 8 batch elements -> 8 NeuronCores, one full
sequence (S=8192) per core. On chip: channels on partitions (2 blocks of
128 for d_inner=256, db-major along the free dim), time on the free dim.

v2 changes vs v1:
  - x activations live in DRAM ping-ping buffers (one per layer); each
    chunk loads a [128, L+3] input tile and stores a [128, L] output tile.
  - B/C broadcasts: xd rows bounce through DRAM once per chunk, then come
    back as FOUR grouped broadcast reads per side ([128, 4, L], 8KB per
    partition descriptors) instead of 32 per-row 2KB-descriptor DMAs.
  - Scan carries: the last h column of each (state, db) is copied by the
    Scalar engine into a small [128, 32] f32 carry tile (was: 512 strided
    DVE CASTs at 1.35us each); next chunk's scans use it as `initial`.
  - Elementwise mults/adds split DVE/GPSIMD by a tunable pattern.
  - ACT ops emitted silu-block-first then exp/ln-block to minimise
    activation-table switches.
"""

import numpy as np

import concourse.bass as bass
import concourse.tile as tile
from concourse import bacc, mybir

F32 = mybir.dt.float32
BF16 = mybir.dt.bfloat16
AF = mybir.ActivationFunctionType
OP = mybir.AluOpType

S = 8192
DM = 128      # d_model
DI = 256      # d_inner
NST = 16      # d_state
DC = 4        # d_conv
RK = 8        # dt_rank
NL = 4        # layers
NCORES = 8

# engine split tunables: which states' ops go to GPSIMD.
# GPSIMD runs ~5.9us per [128,2048] TT (vs 1.2us DVE) and its SBUF-port
# traffic slows concurrent DVE ops, so keep it off the per-state critical
# chain (dbu -> scan -> tm): it only takes accumulator adds + frontend ops.
DBU_GPS = set()
TM_GPS = set()
ADD_GPS = set()


def _patch_act_tables():
    """Steer the greedy ACT-table-set chooser so softplus' exp/ln and the
    per-state da exps all resolve to natural_log_exp_and_others (one table
    set) instead of ping-ponging exp_and_others <-> natural_log.  Only the
    chooser's view is patched; emitted act_func_set_ids still index real
    sets that contain the functions, so runtime tables are correct."""
    from concourse import bacc as _bacc
    if getattr(_bacc, "_act_tables_patched", False):
        return
    _orig = _bacc.get_activation_tables

    def _patched(module_arch):
        t = _orig(module_arch)
        out = {}
        for name, funcs in t.items():
            f = set(funcs)
            if name == "exp_and_others":
                f.discard(AF.Exp)
            if name == "natural_log":
                f.discard(AF.Ln)
            out[name] = f
        return out

    _bacc.get_activation_tables = _patched
    _bacc._act_tables_patched = True


class Ctx:
    pass


def _bcast_view(t, r, L):
    """[128, 2, L] stride-0-partition-dup view of group tile row r."""
    return bass.AP(tensor=t.tensor, offset=t.offset + r * L,
                   ap=[list(t.ap[0]), [0, 2], [1, L]])


def _load_weights(c, nc, drams):
    wp = c.wp
    (kuc, wz, xpw, dtw, ow, emb, headw, dtb, cb, aneg, dpar, embb,
     headb) = drams
    c.w_kuc, c.w_wz, c.w_xpw, c.w_dtw, c.w_ow = [], [], [], [], []
    c.w_dtb, c.w_cb, c.w_a, c.w_d = [], [], [], []
    for l in range(NL):
        for lst, dram, shape, dt in [
            (c.w_kuc, kuc, [128, DC * DI], BF16),
            (c.w_wz, wz, [128, DI], BF16),
            (c.w_xpw, xpw, [128, 80], BF16),
            (c.w_dtw, dtw, [RK, DI], BF16),
            (c.w_ow, ow, [128, 256], BF16),
            (c.w_dtb, dtb, [128, 2], F32),
            (c.w_cb, cb, [128, 2], F32),
            (c.w_a, aneg, [128, 2 * NST], F32),
            (c.w_d, dpar, [128, 2], F32),
        ]:
            t = wp.tile(shape, dt, tag=f"w{len(lst)}_{id(lst) % 997}",
                        name=f"w{len(lst)}_{id(lst) % 997}")
            nc.sync.dma_start(t, dram[l])
            lst.append(t)
    c.w_emb = wp.tile([11, DM], BF16, tag="emb", name="emb")
    nc.sync.dma_start(c.w_emb, emb[:])
    c.w_headw = wp.tile([DM, 1], BF16, tag="headw", name="headw")
    nc.sync.dma_start(c.w_headw, headw[:])
    c.w_embb = wp.tile([128, 1], F32, tag="embb", name="embb")
    nc.sync.dma_start(c.w_embb, embb[:])
    c.w_headb = wp.tile([1, 1], F32, tag="headb", name="headb")
    nc.sync.dma_start(c.w_headb, headb[:])


def _embed(c, nc, featT, xbuf):
    L = c.L
    for blk in range(c.s // L):
        ps = c.pp.tile([128, L], F32, tag="mm", name="mm")
        for h2 in range(L // 512):
            col = blk * L + h2 * 512
            nc.tensor.matmul(
                ps[:, h2 * 512:(h2 + 1) * 512],
                lhsT=c.w_emb, rhs=c.w_feat[:, col:col + 512],
                start=True, stop=True)
        xo = c.xop.tile([128, L], BF16, tag="xo", name="xo")
        nc.scalar.activation(xo, ps, AF.Identity, bias=c.w_embb[:, 0:1])
        nc.sync.dma_start(xbuf[0][:, 3 + blk * L: 3 + (blk + 1) * L], xo)


def _front(c, nc, l, ci, xbuf, bcd):
    """Chunk frontend: projections, conv, silu, x_dbl, softplus, broadcasts.
    Emitted one chunk ahead of the state loop so its serial latency chain
    (and ACT table loads) hide under the previous chunk's scan work."""
    L = c.L
    t0 = ci * L
    fr = Ctx()

    xt = c.xtp.tile([128, L + 3], BF16, tag="xt", name="xt")
    nc.sync.dma_start(xt, xbuf[l][:, t0: t0 + L + 3])

    u = c.up.tile([128, 2 * L], BF16, tag="u", name="u")
    z = c.zp.tile([128, 2 * L], BF16, tag="z", name="z")
    de = c.dep.tile([128, 2 * L], BF16, tag="de", name="de")
    du = c.dup.tile([128, 2 * L], BF16, tag="du", name="du")

    # ---- u path (conv folded into 4 accumulated matmuls) + silu
    for db in range(2):
        ps = c.pp.tile([128, L], F32, tag="mm", name="mm")
        for h2 in range(L // 512):
            for k in range(DC):
                nc.tensor.matmul(
                    ps[:, h2 * 512:(h2 + 1) * 512],
                    lhsT=c.w_kuc[l][:, k * DI + db * 128:k * DI + db * 128 + 128],
                    rhs=xt[:, h2 * 512 + k: h2 * 512 + k + 512],
                    start=(k == 0), stop=(k == DC - 1))
        nc.scalar.activation(u[:, db * L:(db + 1) * L], ps, AF.Silu,
                             bias=c.w_cb[l][:, db:db + 1])
    # ---- z path + silu
    for db in range(2):
        ps = c.pp.tile([128, L], F32, tag="mm", name="mm")
        for h2 in range(L // 512):
            nc.tensor.matmul(
                ps[:, h2 * 512:(h2 + 1) * 512],
                lhsT=c.w_wz[l][:, db * 128:db * 128 + 128],
                rhs=xt[:, 3 + h2 * 512: 3 + h2 * 512 + 512],
                start=True, stop=True)
        nc.scalar.activation(z[:, db * L:(db + 1) * L], ps, AF.Silu)

    # ---- x_dbl = u @ xp_w -> [40, L]
    xd = c.xdp.tile([40, L], BF16, tag="xd", name="xd")
    ps = c.pp.tile([128, L], F32, tag="mm", name="mm")
    for h2 in range(L // 512):
        for ct in range(2):
            nc.tensor.matmul(
                ps[0:40, h2 * 512:(h2 + 1) * 512],
                lhsT=c.w_xpw[l][:, ct * 40:ct * 40 + 40],
                rhs=u[:, ct * L + h2 * 512: ct * L + h2 * 512 + 512],
                start=(ct == 0), stop=(ct == 1))
    nc.scalar.activation(xd, ps[0:40, :], AF.Copy)

    # ---- B/C rows to DRAM, back as 4-row grouped partition-broadcasts
    slot = (l * c.nch + ci) % 4
    nc.gpsimd.dma_start(bcd[slot], xd[RK:RK + 2 * NST, :])
    fr.bgrp, fr.cgrp = [], []
    for g in range(4):
        bb = c.bbp.tile([128, 4, L], BF16, tag="bb", name="bb")
        grp = bcd[slot][4 * g:4 * g + 4, :]
        src = bass.AP(tensor=grp.tensor, offset=grp.offset,
                      ap=[[0, 128]] + [list(x) for x in grp.ap])
        (nc.sync if g % 2 == 0 else nc.scalar).dma_start(bb, src)
        fr.bgrp.append(bb)
    for g in range(4):
        cc = c.ccp.tile([128, 4, L], BF16, tag="cc", name="cc")
        grp = bcd[slot][NST + 4 * g:NST + 4 * g + 4, :]
        src = bass.AP(tensor=grp.tensor, offset=grp.offset,
                      ap=[[0, 128]] + [list(x) for x in grp.ap])
        (nc.scalar if g % 2 == 0 else nc.sync).dma_start(cc, src)
        fr.cgrp.append(cc)

    # ---- delta = softplus(dt @ dtw + dtb), both db, set-batched ACT ops
    # softplus(x) = relu(x) + ln(1 + exp(-|x|))
    pss, abs_, ab2s = [], [], []
    for db in range(2):
        ps = c.pp.tile([128, L], F32, tag="mm", name="mm")
        for h2 in range(L // 512):
            nc.tensor.matmul(
                ps[:, h2 * 512:(h2 + 1) * 512],
                lhsT=c.w_dtw[l][:, db * 128:db * 128 + 128],
                rhs=xd[0:RK, h2 * 512:h2 * 512 + 512],
                start=True, stop=True)
        pss.append(ps)
    for db in range(2):
        ab = c.abp.tile([128, L], F32, tag="ab", name="ab")
        nc.scalar.activation(ab, pss[db], AF.Abs,
                             bias=c.w_dtb[l][:, db:db + 1])
        abs_.append(ab)
    for db in range(2):
        nc.scalar.activation(de[:, db * L:(db + 1) * L], pss[db], AF.Relu,
                             bias=c.w_dtb[l][:, db:db + 1])
    for db in range(2):
        ab2 = c.ab2p.tile([128, L], BF16, tag="ab2", name="ab2")
        nc.scalar.activation(ab2, abs_[db], AF.Exp, scale=-1.0)
        ab2s.append(ab2)
    for db in range(2):
        nc.scalar.activation(ab2s[db], ab2s[db], AF.Ln, bias=1.0)
    for db in range(2):
        nc.vector.tensor_tensor(de[:, db * L:(db + 1) * L],
                                de[:, db * L:(db + 1) * L], ab2s[db], OP.add)

    nc.vector.tensor_tensor(du, de, u, OP.mult)
    fr.u, fr.z, fr.de, fr.du = u, z, de, du
    return fr


def _states(c, nc, l, ci, fr, xbuf):
    L = c.L
    t0 = ci * L
    u, z, de, du = fr.u, fr.z, fr.de, fr.du

    # ---- selective scan over 16 states
    hc = c.hcp.tile([128, 2 * NST], F32, tag="hc", name="hc")
    acc = [None, None]
    de3 = de.rearrange("p (b t) -> p b t", b=2)
    du3 = du.rearrange("p (b t) -> p b t", b=2)
    for n in range(NST):
        g, r = n // 4, n % 4
        # da = exp(delta * A_n) per db (f32 for scan precision)
        das = []
        for db in range(2):
            da = c.dap.tile([128, L], F32, tag="da", name="da")
            nc.scalar.activation(
                da, de3[:, db, :], AF.Exp,
                scale=c.w_a[l][:, db * NST + n:db * NST + n + 1])
            das.append(da)
        # dbu = du * B_n (B broadcast across partitions, dup over db)
        dbu = c.dbup.tile([128, 2 * L], BF16, tag="dbu", name="dbu")
        deng = nc.gpsimd if n in DBU_GPS else nc.vector
        deng.tensor_tensor(dbu.rearrange("p (b t) -> p b t", b=2), du3,
                           _bcast_view(fr.bgrp[g], r, L), OP.mult)
        # scan: h = da * h_prev + dbu, chained via carry tile
        h = c.hp.tile([128, 2 * L], BF16, tag="h", name="h")
        for db in range(2):
            init = 0.0 if ci == 0 else c.hc_prev[:, 2 * n + db:2 * n + db + 1]
            nc.vector.tensor_tensor_scan(
                h[:, db * L:(db + 1) * L], das[db],
                dbu[:, db * L:(db + 1) * L],
                initial=init, op0=OP.mult, op1=OP.add)
        # carry out last column of both db segments (Scalar engine copy)
        hsrc = bass.AP(tensor=h.tensor, offset=h.offset + L - 1,
                       ap=[list(h.ap[0]), [L, 2]])
        nc.scalar.activation(hc[:, 2 * n:2 * n + 2], hsrc, AF.Copy)
        # tm = h * C_n ; accumulate into acc[n % 2]
        teng = nc.gpsimd if n in TM_GPS else nc.vector
        if n < 2:
            a = c.accp.tile([128, 2 * L], BF16, tag=f"acc{n}", name="acc")
            teng.tensor_tensor(a.rearrange("p (b t) -> p b t", b=2),
                               h.rearrange("p (b t) -> p b t", b=2),
                               _bcast_view(fr.cgrp[g], r, L), OP.mult)
            acc[n] = a
        else:
            tm = c.tmp.tile([128, 2 * L], BF16, tag="tm", name="tm")
            teng.tensor_tensor(tm.rearrange("p (b t) -> p b t", b=2),
                               h.rearrange("p (b t) -> p b t", b=2),
                               _bcast_view(fr.cgrp[g], r, L), OP.mult)
            aeng = nc.gpsimd if n in ADD_GPS else nc.vector
            aeng.tensor_tensor(acc[n % 2], acc[n % 2], tm, OP.add)
    c.hc_prev = hc

    # ---- y = acc0 + acc1 + u*D, gated by silu(z)
    y = acc[0]
    tmD = c.tmp.tile([128, 2 * L], BF16, tag="tm", name="tmD")
    for db in range(2):
        sl = slice(db * L, (db + 1) * L)
        nc.vector.tensor_scalar(tmD[:, sl], u[:, sl],
                                c.w_d[l][:, db:db + 1], None, OP.mult)
    nc.vector.tensor_tensor(y, y, acc[1], OP.add)
    nc.vector.tensor_tensor(y, y, tmD, OP.add)
    nc.vector.tensor_tensor(y, y, z, OP.mult)

    # ---- out_proj -> next layer's x buffer
    ps = c.pp.tile([128, L], F32, tag="mm", name="mm")
    for h2 in range(L // 512):
        for ct in range(2):
            nc.tensor.matmul(
                ps[:, h2 * 512:(h2 + 1) * 512],
                lhsT=c.w_ow[l][:, ct * 128:ct * 128 + 128],
                rhs=y[:, ct * L + h2 * 512: ct * L + h2 * 512 + 512],
                start=(ct == 0), stop=(ct == 1))
    xo = c.xop.tile([128, L], BF16, tag="xo", name="xo")
    nc.scalar.activation(xo, ps, AF.Copy)
    nc.scalar.dma_start(xbuf[l + 1][:, 3 + t0: 3 + t0 + L], xo)


def _head(c, nc, xbuf, out):
    L = c.L
    for blk in range(c.s // L):
        xt = c.xtp.tile([128, L + 3], BF16, tag="xt", name="xt")
        nc.sync.dma_start(xt, xbuf[NL][:, blk * L: blk * L + L + 3])
        ps = c.pp.tile([128, L], F32, tag="mm", name="mm")
        for h2 in range(L // 512):
            nc.tensor.matmul(
                ps[0:1, h2 * 512:(h2 + 1) * 512],
                lhsT=c.w_headw, rhs=xt[:, 3 + h2 * 512: 3 + h2 * 512 + 512],
                start=True, stop=True)
        ot = c.abp.tile([128, L], F32, tag="ab", name="ot")
        nc.scalar.activation(ot[0:1, :], ps[0:1, :], AF.Sigmoid,
                             bias=c.w_headb[0:1, 0:1])
        nc.sync.dma_start(out[0:1, blk * L:(blk + 1) * L], ot[0:1, :])


def build(s=S, tc_len=1024, nloops=1):
    _patch_act_tables()
    nc = bacc.Bacc("TRN2", target_bir_lowering=False, debug=False,
                   num_devices=NCORES)
    drams = (
        nc.declare_dram_parameter("kuc", [NL, 128, DC * DI], BF16, False),
        nc.declare_dram_parameter("wz", [NL, 128, DI], BF16, False),
        nc.declare_dram_parameter("xpw", [NL, 128, 80], BF16, False),
        nc.declare_dram_parameter("dtw", [NL, RK, DI], BF16, False),
        nc.declare_dram_parameter("ow", [NL, 128, 256], BF16, False),
        nc.declare_dram_parameter("emb", [11, DM], BF16, False),
        nc.declare_dram_parameter("headw", [DM, 1], BF16, False),
        nc.declare_dram_parameter("dtb", [NL, 128, 2], F32, False),
        nc.declare_dram_parameter("cb", [NL, 128, 2], F32, False),
        nc.declare_dram_parameter("aneg", [NL, 128, 2 * NST], F32, False),
        nc.declare_dram_parameter("dpar", [NL, 128, 2], F32, False),
        nc.declare_dram_parameter("embb", [128, 1], F32, False),
        nc.declare_dram_parameter("headb", [1, 1], F32, False),
    )
    featT = nc.declare_dram_parameter("featT", [11, s], BF16, False)
    out = nc.declare_dram_parameter("out", [1, s], F32, True)
    bcd = nc.dram_tensor("bcd", [4, 2 * NST, tc_len], BF16)
    xbuf = [nc.dram_tensor(f"xb{i}", [128, 3 + s], BF16)
            for i in range(NL + 1)]

    c = Ctx()
    c.s = s
    c.L = tc_len
    c.nch = s // tc_len

    with tile.TileContext(nc) as tcx:
        with (
            tcx.tile_pool(name="w", bufs=1) as wp,
            tcx.tile_pool(name="psP", bufs=3, space="PSUM") as pp,
            tcx.tile_pool(name="xo", bufs=3) as xop,
            tcx.tile_pool(name="ab", bufs=2) as abp,
        ):
            c.wp, c.pp, c.xop, c.abp = wp, pp, xop, abp
            _load_weights(c, nc, drams)

            zt = wp.tile([128, 3], BF16, tag="zpad", name="zpad")
            nc.vector.memset(zt, 0.0)
            for i in range(NL + 1):
                nc.sync.dma_start(xbuf[i][:, 0:3], zt)

            with tcx.tile_pool(name="feat", bufs=1) as fp:
                c.w_feat = fp.tile([11, s], BF16, tag="featT", name="featT")
                nc.sync.dma_start(c.w_feat, featT[:])
                _embed(c, nc, featT, xbuf)

            from contextlib import ExitStack
            with ExitStack() as es:
                pools = {
                    "xtp": ("xt", 3), "up": ("u", 2), "zp": ("z", 2),
                    "dep": ("de", 2), "dup": ("du", 2), "xdp": ("xd", 2),
                    "bbp": ("bb", 2), "ccp": ("cc", 2), "dap": ("da", 4),
                    "dbup": ("dbu", 4), "hp": ("h", 6), "hcp": ("hcp", 2),
                    "tmp": ("tm", 4), "accp": ("accp", 2),
                    "ab2p": ("ab2", 2),
                }
                for attr, (nm, bufs) in pools.items():
                    setattr(c, attr,
                            es.enter_context(tcx.tile_pool(name=nm, bufs=bufs)))

                chunks = [(l, ci) for rep in range(nloops)
                          for l in range(NL) for ci in range(c.nch)]
                fr = _front(c, nc, chunks[0][0], chunks[0][1], xbuf, bcd)
                for idx, (l, ci) in enumerate(chunks):
                    nxt = chunks[idx + 1] if idx + 1 < len(chunks) else None
                    fr_next = (_front(c, nc, nxt[0], nxt[1], xbuf, bcd)
                               if nxt else None)
                    _states(c, nc, l, ci, fr, xbuf)
                    fr = fr_next
                _head(c, nc, xbuf, out)

    nc.compile()
    return nc


_CACHE = {}


def _get_nc(s, tc_len, nloops=1):
    key = (s, tc_len, nloops)
    if key not in _CACHE:
        _CACHE[key] = build(s, tc_len, nloops)
    return _CACHE[key]


def prep_inputs(features, emb_w, emb_b, in_proj_w, conv_w, conv_b, x_proj_w,
                dt_w, dt_b, A_log, D, out_proj_w, head_w, head_b):
    """Host-side weight preprocessing shared by all cores."""
    import ml_dtypes
    f32 = np.float32
    bf16 = ml_dtypes.bfloat16

    nl = in_proj_w.shape[0]
    kuc = np.zeros((nl, 128, DC * DI), dtype=f32)
    for l in range(nl):
        wu = in_proj_w[l][:, :DI]                      # [128, 256]
        for k in range(DC):
            kuc[l][:, k * DI:(k + 1) * DI] = wu * conv_w[l][:, k][None, :]
    wz = in_proj_w[:, :, DI:]                          # [NL, 128, 256]
    xpw = np.zeros((nl, 128, 80), dtype=f32)
    ow = np.zeros((nl, 128, 256), dtype=f32)
    aneg = np.zeros((nl, 128, 2 * NST), dtype=f32)
    dtb2 = np.zeros((nl, 128, 2), dtype=f32)
    cb2 = np.zeros((nl, 128, 2), dtype=f32)
    dp2 = np.zeros((nl, 128, 2), dtype=f32)
    for l in range(nl):
        for ct in range(2):
            xpw[l][:, ct * 40:(ct + 1) * 40] = \
                x_proj_w[l][ct * 128:(ct + 1) * 128, :]
            ow[l][:, ct * 128:(ct + 1) * 128] = \
                out_proj_w[l][ct * 128:(ct + 1) * 128, :]
            aneg[l][:, ct * NST:(ct + 1) * NST] = \
                -np.exp(A_log[l][ct * 128:(ct + 1) * 128, :])
            dtb2[l][:, ct] = dt_b[l][ct * 128:(ct + 1) * 128]
            cb2[l][:, ct] = conv_b[l][ct * 128:(ct + 1) * 128]
            dp2[l][:, ct] = D[l][ct * 128:(ct + 1) * 128]

    return {
        "kuc": kuc.astype(bf16),
        "wz": np.ascontiguousarray(wz).astype(bf16),
        "xpw": xpw.astype(bf16),
        "dtw": np.ascontiguousarray(dt_w).astype(bf16),
        "ow": ow.astype(bf16),
        "emb": np.ascontiguousarray(emb_w).astype(bf16),
        "headw": np.ascontiguousarray(head_w).astype(bf16),
        "dtb": dtb2,
        "cb": cb2,
        "aneg": aneg,
        "dpar": dp2,
        "embb": np.asarray(emb_b).reshape(128, 1).astype(f32),
        "headb": np.asarray(head_b).reshape(1, 1).astype(f32),
    }


def kernel(features, emb_w, emb_b, in_proj_w, conv_w, conv_b, x_proj_w,
           dt_w, dt_b, A_log, D, out_proj_w, head_w, head_b,
           _tc_len=1024, _trace=False):
    from concourse.bass_utils import run_bass_kernel_spmd
    import ml_dtypes

    args = [np.asarray(a) for a in (
        features, emb_w, emb_b, in_proj_w, conv_w, conv_b, x_proj_w,
        dt_w, dt_b, A_log, D, out_proj_w, head_w, head_b)]
    features = args[0]
    b, s, _ = features.shape
    assert b == NCORES
    nc = _get_nc(s, _tc_len)
    common = prep_inputs(*args)
    in_maps = []
    for i in range(NCORES):
        m = dict(common)
        m["featT"] = np.ascontiguousarray(
            features[i].T).astype(ml_dtypes.bfloat16)
        in_maps.append(m)
    res = run_bass_kernel_spmd(nc, in_maps, core_ids=list(range(NCORES)),
                               trace=_trace)
    out = np.stack([r["out"].reshape(s, 1) for r in res.results])
    kernel.last_result = res
    return out.astype(np.float32)
